# revision 1
# baseline (speedup 1.0000x reference)
"""GTELayer kernel sharded across 8 trn2 NeuronCores.

Strategy (per sharding_hint): shard the dense [N,N,H] score tensor by
destination-node rows (256 rows/core). Softmax over source axis and the
A@V reduction are row-local. Node features / weights are replicated.
Sparse-graph edges are sharded by destination row range on the host
(index prep only); each core gathers sc and scatter-adds pe for its own
rows and produces its slice of e2. The tiny node-FFN tail (needs global
GraphNorm stats over all 2048 nodes) runs as a second single-core jit.
"""

import numpy as np
import jax
import jax.numpy as jnp

N = 2048
H = 8
D = 16
HD = 128
ED = 64
FF = 512
EG = 131072
M = 8            # cores
R = N // M       # dst rows per core

_WNAMES = [
    'gn1_w', 'gn1_b', 'gn1_a', 'ln1e_w', 'ln1e_b',
    'Wq', 'bq', 'Wk', 'bk', 'Wv', 'bv', 'Wpe', 'bpe', 'Wap', 'bap',
    'Wo', 'bo', 'Woe', 'boe', 'gn2_w', 'gn2_b', 'gn2_a', 'ln2e_w', 'ln2e_b',
    'Wh1', 'bh1', 'Wh2', 'bh2', 'We1', 'be1', 'We2', 'be2',
]


def _lin(x, W, b):
    return x @ W + b


def _graph_norm(x, w, b, a):
    mu = jnp.mean(x, axis=0, keepdims=True)
    sub = x - a * mu
    var = jnp.mean(sub * sub, axis=0, keepdims=True)
    return w * sub / jnp.sqrt(var + 1e-5) + b


def _layer_norm(x, w, b):
    mu = jnp.mean(x, axis=-1, keepdims=True)
    var = jnp.var(x, axis=-1, keepdims=True)
    return w * (x - mu) / jnp.sqrt(var + 1e-5) + b


def _ffn(x, W1, b1, W2, b2):
    return _lin(jax.nn.gelu(_lin(x, W1, b1), approximate=False), W2, b2)


def _core_fn(w, x, row_start, adj2_rows, rel_pos_rows, e_rows,
             src_e, dstloc_e, valid_e):
    """Work for one core: 256 dst rows of attention + its edge slice."""
    y = _graph_norm(x, w['gn1_w'], w['gn1_b'], w['gn1_a'])
    Q = _lin(y, w['Wq'], w['bq']).reshape(N, H, D)
    K = _lin(y, w['Wk'], w['bk']).reshape(N, H, D)
    V = _lin(y, w['Wv'], w['bv']).reshape(N, H, D)

    Qr = jax.lax.dynamic_slice_in_dim(Q, row_start, R, axis=0)   # [R,H,D]

    en = _layer_norm(e_rows, w['ln1e_w'], w['ln1e_b'])           # [C,ED]
    pe = _lin(en, w['Wpe'], w['bpe'])                            # [C,H]

    S = jnp.einsum('jhd,ihd->ijh', K, Qr) / np.float32(np.sqrt(D))
    S = S * adj2_rows[..., None] + rel_pos_rows                  # [R,N,H]

    sc = S[dstloc_e, src_e]                                      # [C,H]
    e_out = _lin(sc, w['Wap'], w['bap'])                         # [C,ED]
    # set(sc + pe) at unique edges == add(pe); padding adds 0 -> safe dups
    S = S.at[dstloc_e, src_e].add(pe * valid_e[:, None])

    A = jax.nn.softmax(jnp.clip(S, -5.0, 5.0), axis=1)
    wV = jnp.einsum('ijh,jhd->ihd', A, V).reshape(R, HD)
    h_out = _lin(wV, w['Wo'], w['bo'])                           # [R,HD]

    e_out = _lin(e_out + en, w['Woe'], w['boe'])
    e2 = e_rows + e_out
    e2 = e2 + _ffn(_layer_norm(e2, w['ln2e_w'], w['ln2e_b']),
                   w['We1'], w['be1'], w['We2'], w['be2'])
    return h_out, e2


def _tail_fn(w, x, h_out):
    x2 = x + h_out
    x2 = x2 + _ffn(_graph_norm(x2, w['gn2_w'], w['gn2_b'], w['gn2_a']),
                   w['Wh1'], w['bh1'], w['Wh2'], w['bh2'])
    return x2


_pmapped = jax.pmap(
    _core_fn,
    in_axes=(None, None, 0, 0, 0, 0, 0, 0, 0),
)
_tail_jit = jax.jit(_tail_fn)


def kernel(**inputs):
    x = inputs['x'].astype(np.float32)
    e = inputs['e'].astype(np.float32)
    adj2 = inputs['adj2'].astype(np.float32)
    rel_pos = inputs['rel_pos'].astype(np.float32)
    src = np.asarray(inputs['src'], dtype=np.int32)
    dst = np.asarray(inputs['dst'], dtype=np.int32)
    w = {k: jnp.asarray(inputs[k], dtype=jnp.float32) for k in _WNAMES}

    # --- host-side shard prep (index bookkeeping only) ---
    core_of_edge = dst // R
    idx_per_core = [np.nonzero(core_of_edge == c)[0] for c in range(M)]
    C = max(len(ix) for ix in idx_per_core)
    C = int(-(-C // 128) * 128)  # pad to a multiple of 128

    e_rows = np.zeros((M, C, ED), dtype=np.float32)
    src_e = np.zeros((M, C), dtype=np.int32)
    dstloc_e = np.zeros((M, C), dtype=np.int32)
    valid_e = np.zeros((M, C), dtype=np.float32)
    for c, ix in enumerate(idx_per_core):
        n = len(ix)
        e_rows[c, :n] = e[ix]
        src_e[c, :n] = src[ix]
        dstloc_e[c, :n] = dst[ix] - c * R
        valid_e[c, :n] = 1.0

    row_start = np.arange(M, dtype=np.int32) * R
    adj2_sh = adj2.reshape(M, R, N)
    rel_pos_sh = rel_pos.reshape(M, R, N, H)

    h_out_sh, e2_sh = _pmapped(
        w, jnp.asarray(x), jnp.asarray(row_start),
        jnp.asarray(adj2_sh), jnp.asarray(rel_pos_sh), jnp.asarray(e_rows),
        jnp.asarray(src_e), jnp.asarray(dstloc_e), jnp.asarray(valid_e),
    )
    h_out = np.asarray(h_out_sh).reshape(N, HD)
    x2 = np.asarray(_tail_jit(w, jnp.asarray(x), jnp.asarray(h_out)))

    e2 = np.empty((EG, ED), dtype=np.float32)
    e2_sh = np.asarray(e2_sh)
    for c, ix in enumerate(idx_per_core):
        e2[ix] = e2_sh[c, :len(ix)]

    return np.asarray(x2, dtype=np.float32), e2


# revision 2
# speedup vs baseline: 26.0792x; 26.0792x over previous
"""GTELayer kernel sharded across 8 trn2 NeuronCores.

Strategy (per sharding_hint): shard the dense [N,N,H] score tensor by
destination-node rows (256 rows/core). Softmax over source axis and the
A@V reduction are row-local. Node features / weights are replicated.
Sparse-graph edges are sharded by destination row range on the host
(index prep only); each core gathers sc and scatter-adds pe for its own
rows and produces its slice of e2. GraphNorm-2 global stats are formed
with an 8-way psum of per-core partial sums, so the whole layer is a
single pmap program. Device-side score layout is [i, h, j] (h-major) so
softmax reduces over the contiguous axis and no device transposes of the
[N,N,H] tensor are needed (rel_pos is pre-transposed once on host).
"""

import numpy as np
import jax
import jax.numpy as jnp
from jax import lax

try:
    jax.config.update("jax_compilation_cache_dir", "/tmp/jax_neuron_cache")
    jax.config.update("jax_persistent_cache_min_compile_time_secs", 1.0)
except Exception:
    pass

N = 2048
H = 8
D = 16
HD = 128
ED = 64
FF = 512
EG = 131072
M = 8            # cores
R = N // M       # dst rows per core

_WNAMES = [
    'gn1_w', 'gn1_b', 'gn1_a', 'ln1e_w', 'ln1e_b',
    'Wq', 'bq', 'Wk', 'bk', 'Wv', 'bv', 'Wpe', 'bpe', 'Wap', 'bap',
    'Wo', 'bo', 'Woe', 'boe', 'gn2_w', 'gn2_b', 'gn2_a', 'ln2e_w', 'ln2e_b',
    'Wh1', 'bh1', 'Wh2', 'bh2', 'We1', 'be1', 'We2', 'be2',
]


def _lin(x, W, b):
    return x @ W + b


def _graph_norm(x, w, b, a):
    mu = jnp.mean(x, axis=0, keepdims=True)
    sub = x - a * mu
    var = jnp.mean(sub * sub, axis=0, keepdims=True)
    return w * sub / jnp.sqrt(var + 1e-5) + b


def _layer_norm(x, w, b):
    mu = jnp.mean(x, axis=-1, keepdims=True)
    var = jnp.var(x, axis=-1, keepdims=True)
    return w * (x - mu) / jnp.sqrt(var + 1e-5) + b


def _ffn(x, W1, b1, W2, b2):
    return _lin(jax.nn.gelu(_lin(x, W1, b1), approximate=False), W2, b2)


def _core_fn(w, x, row_start, adj2_rows, rel_pos_t, e_rows,
             src_e, dstloc_e, valid_e):
    """One core: 256 dst rows of attention + its edge slice + node tail."""
    y = _graph_norm(x, w['gn1_w'], w['gn1_b'], w['gn1_a'])
    Q = _lin(y, w['Wq'], w['bq']).reshape(N, H, D)
    K = _lin(y, w['Wk'], w['bk']).reshape(N, H, D)
    V = _lin(y, w['Wv'], w['bv']).reshape(N, H, D)

    xr = lax.dynamic_slice_in_dim(x, row_start, R, axis=0)       # [R,HD]
    Qr = lax.dynamic_slice_in_dim(Q, row_start, R, axis=0)       # [R,H,D]

    en = _layer_norm(e_rows, w['ln1e_w'], w['ln1e_b'])           # [C,ED]
    pe = _lin(en, w['Wpe'], w['bpe'])                            # [C,H]

    # S in [i, h, j] layout: softmax axis is contiguous, no transposes
    S = jnp.einsum('jhd,ihd->ihj', K, Qr) / np.float32(np.sqrt(D))
    S = S * adj2_rows[:, None, :] + rel_pos_t                    # [R,H,N]

    sc = S[dstloc_e, :, src_e]                                   # [C,H]
    e_out = _lin(sc, w['Wap'], w['bap'])                         # [C,ED]
    # set(sc + pe) at unique edges == add(pe); padding adds 0 -> safe dups
    S = S.at[dstloc_e, :, src_e].add(pe * valid_e[:, None])

    A = jax.nn.softmax(jnp.clip(S, -5.0, 5.0), axis=-1)          # [R,H,N]
    wV = jnp.einsum('ihj,jhd->ihd', A, V).reshape(R, HD)
    h_out = _lin(wV, w['Wo'], w['bo'])                           # [R,HD]

    # edge branch tail (fully core-local)
    e_out = _lin(e_out + en, w['Woe'], w['boe'])
    e2 = e_rows + e_out
    e2 = e2 + _ffn(_layer_norm(e2, w['ln2e_w'], w['ln2e_b']),
                   w['We1'], w['be1'], w['We2'], w['be2'])

    # node tail: GraphNorm over ALL nodes via psum of partial sums
    x2 = xr + h_out                                              # [R,HD]
    s1 = lax.psum(jnp.sum(x2, axis=0), axis_name='c')            # [HD]
    s2 = lax.psum(jnp.sum(x2 * x2, axis=0), axis_name='c')       # [HD]
    a = w['gn2_a']
    mu = s1 / np.float32(N)
    var = s2 / np.float32(N) - (2.0 * a - a * a) * mu * mu
    gn = w['gn2_w'] * (x2 - a * mu) / jnp.sqrt(var + 1e-5) + w['gn2_b']
    x2 = x2 + _ffn(gn, w['Wh1'], w['bh1'], w['Wh2'], w['bh2'])
    return x2, e2


_pmapped = jax.pmap(
    _core_fn,
    axis_name='c',
    in_axes=(None, None, 0, 0, 0, 0, 0, 0, 0),
)


def kernel(**inputs):
    x = inputs['x'].astype(np.float32)
    e = inputs['e'].astype(np.float32)
    adj2 = inputs['adj2'].astype(np.float32)
    rel_pos = inputs['rel_pos'].astype(np.float32)
    src = np.asarray(inputs['src'], dtype=np.int32)
    dst = np.asarray(inputs['dst'], dtype=np.int32)
    w = {k: jnp.asarray(inputs[k], dtype=jnp.float32) for k in _WNAMES}

    # --- host-side shard prep (index bookkeeping only) ---
    core_of_edge = dst // R
    idx_per_core = [np.nonzero(core_of_edge == c)[0] for c in range(M)]
    C = max(len(ix) for ix in idx_per_core)
    C = int(-(-C // 128) * 128)  # pad to a multiple of 128

    e_rows = np.zeros((M, C, ED), dtype=np.float32)
    src_e = np.zeros((M, C), dtype=np.int32)
    dstloc_e = np.zeros((M, C), dtype=np.int32)
    valid_e = np.zeros((M, C), dtype=np.float32)
    for c, ix in enumerate(idx_per_core):
        n = len(ix)
        e_rows[c, :n] = e[ix]
        src_e[c, :n] = src[ix]
        dstloc_e[c, :n] = dst[ix] - c * R
        valid_e[c, :n] = 1.0

    row_start = np.arange(M, dtype=np.int32) * R
    adj2_sh = adj2.reshape(M, R, N)
    # [M, R, N, H] -> [M, R, H, N] once on host
    rel_pos_t = np.ascontiguousarray(
        rel_pos.reshape(M, R, N, H).transpose(0, 1, 3, 2))

    x2_sh, e2_sh = _pmapped(
        w, jnp.asarray(x), jnp.asarray(row_start),
        jnp.asarray(adj2_sh), jnp.asarray(rel_pos_t), jnp.asarray(e_rows),
        jnp.asarray(src_e), jnp.asarray(dstloc_e), jnp.asarray(valid_e),
    )

    x2 = np.asarray(x2_sh).reshape(N, HD).astype(np.float32)
    e2 = np.empty((EG, ED), dtype=np.float32)
    e2_sh = np.asarray(e2_sh)
    for c, ix in enumerate(idx_per_core):
        e2[ix] = e2_sh[c, :len(ix)]

    return x2, e2


# revision 4
# speedup vs baseline: 30.0473x; 1.1522x over previous
"""GTELayer kernel sharded across 8 trn2 NeuronCores.

Strategy (per sharding_hint): shard the dense [N,N,H] score tensor by
destination-node rows (256 rows/core). Softmax over source axis and the
A@V reduction are row-local. Node features / weights are replicated.
Sparse-graph edges are sharded by destination row range on the host
(index prep only); each core gathers sc and scatter-adds pe for its own
rows and produces its slice of e2. GraphNorm-2 global stats are formed
with an 8-way psum of per-core partial sums, so the whole layer is a
single pmap program. Device-side score layout is [i, h, j] (h-major) so
softmax reduces over the contiguous axis and no device transposes of the
[N,N,H] tensor are needed (rel_pos is pre-transposed once on host).
"""

import numpy as np
import jax
import jax.numpy as jnp
from jax import lax

try:
    jax.config.update("jax_compilation_cache_dir", "/tmp/jax_neuron_cache")
    jax.config.update("jax_persistent_cache_min_compile_time_secs", 1.0)
except Exception:
    pass

N = 2048
H = 8
D = 16
HD = 128
ED = 64
FF = 512
EG = 131072
M = 8            # cores
R = N // M       # dst rows per core

_WNAMES = [
    'gn1_w', 'gn1_b', 'gn1_a', 'ln1e_w', 'ln1e_b',
    'Wq', 'bq', 'Wk', 'bk', 'Wv', 'bv', 'Wpe', 'bpe', 'Wap', 'bap',
    'Wo', 'bo', 'Woe', 'boe', 'gn2_w', 'gn2_b', 'gn2_a', 'ln2e_w', 'ln2e_b',
    'Wh1', 'bh1', 'Wh2', 'bh2', 'We1', 'be1', 'We2', 'be2',
]


def _lin(x, W, b):
    return x @ W + b


def _graph_norm(x, w, b, a):
    mu = jnp.mean(x, axis=0, keepdims=True)
    sub = x - a * mu
    var = jnp.mean(sub * sub, axis=0, keepdims=True)
    return w * sub / jnp.sqrt(var + 1e-5) + b


def _layer_norm(x, w, b):
    mu = jnp.mean(x, axis=-1, keepdims=True)
    var = jnp.var(x, axis=-1, keepdims=True)
    return w * (x - mu) / jnp.sqrt(var + 1e-5) + b


def _ffn(x, W1, b1, W2, b2):
    return _lin(jax.nn.gelu(_lin(x, W1, b1), approximate=False), W2, b2)


def _core_fn(w, x, row_start, adj2_rows, rel_pos_t, e_rows,
             src_e, dstloc_e, valid_e):
    """One core: 256 dst rows of attention + its edge slice + node tail."""
    y = _graph_norm(x, w['gn1_w'], w['gn1_b'], w['gn1_a'])
    Q = _lin(y, w['Wq'], w['bq']).reshape(N, H, D)
    K = _lin(y, w['Wk'], w['bk']).reshape(N, H, D)
    V = _lin(y, w['Wv'], w['bv']).reshape(N, H, D)

    xr = lax.dynamic_slice_in_dim(x, row_start, R, axis=0)       # [R,HD]
    Qr = lax.dynamic_slice_in_dim(Q, row_start, R, axis=0)       # [R,H,D]

    en = _layer_norm(e_rows, w['ln1e_w'], w['ln1e_b'])           # [C,ED]
    pe = _lin(en, w['Wpe'], w['bpe'])                            # [C,H]

    # S in [i, h, j] layout: softmax axis is contiguous, no transposes
    S = jnp.einsum('jhd,ihd->ihj', K, Qr) / np.float32(np.sqrt(D))
    S = S * adj2_rows[:, None, :] + rel_pos_t                    # [R,H,N]

    sc = S[dstloc_e, :, src_e]                                   # [C,H]
    e_out = _lin(sc, w['Wap'], w['bap'])                         # [C,ED]
    # set(sc + pe) at unique edges == add(pe); padding adds 0 -> safe dups
    S = S.at[dstloc_e, :, src_e].add(pe * valid_e[:, None])

    A = jax.nn.softmax(jnp.clip(S, -5.0, 5.0), axis=-1)          # [R,H,N]
    wV = jnp.einsum('ihj,jhd->ihd', A, V).reshape(R, HD)
    h_out = _lin(wV, w['Wo'], w['bo'])                           # [R,HD]

    # edge branch tail (fully core-local)
    e_out = _lin(e_out + en, w['Woe'], w['boe'])
    e2 = e_rows + e_out
    e2 = e2 + _ffn(_layer_norm(e2, w['ln2e_w'], w['ln2e_b']),
                   w['We1'], w['be1'], w['We2'], w['be2'])

    # node tail: GraphNorm over ALL nodes via psum of partial sums
    x2 = xr + h_out                                              # [R,HD]
    s1 = lax.psum(jnp.sum(x2, axis=0), axis_name='c')            # [HD]
    s2 = lax.psum(jnp.sum(x2 * x2, axis=0), axis_name='c')       # [HD]
    a = w['gn2_a']
    mu = s1 / np.float32(N)
    var = s2 / np.float32(N) - (2.0 * a - a * a) * mu * mu
    gn = w['gn2_w'] * (x2 - a * mu) / jnp.sqrt(var + 1e-5) + w['gn2_b']
    x2 = x2 + _ffn(gn, w['Wh1'], w['bh1'], w['Wh2'], w['bh2'])
    return x2, e2


_pmapped = jax.pmap(
    _core_fn,
    axis_name='c',
    in_axes=(None, None, 0, 0, 0, 0, 0, 0, 0),
)


def kernel(**inputs):
    x = inputs['x'].astype(np.float32)
    e = inputs['e'].astype(np.float32)
    adj2 = inputs['adj2'].astype(np.float32)
    rel_pos = inputs['rel_pos'].astype(np.float32)
    src = np.asarray(inputs['src'], dtype=np.int32)
    dst = np.asarray(inputs['dst'], dtype=np.int32)
    w = {k: jnp.asarray(inputs[k], dtype=jnp.float32) for k in _WNAMES}

    # --- host-side shard prep (index bookkeeping only) ---
    core_of_edge = dst // R
    idx_per_core = [np.nonzero(core_of_edge == c)[0] for c in range(M)]
    C = max(len(ix) for ix in idx_per_core)
    C = int(-(-C // 128) * 128)  # pad to a multiple of 128

    e_rows = np.zeros((M, C, ED), dtype=np.float32)
    src_e = np.zeros((M, C), dtype=np.int32)
    dstloc_e = np.zeros((M, C), dtype=np.int32)
    valid_e = np.zeros((M, C), dtype=np.float32)
    for c, ix in enumerate(idx_per_core):
        n = len(ix)
        e_rows[c, :n] = e[ix]
        src_e[c, :n] = src[ix]
        dstloc_e[c, :n] = dst[ix] - c * R
        valid_e[c, :n] = 1.0

    row_start = np.arange(M, dtype=np.int32) * R
    adj2_sh = adj2.reshape(M, R, N)
    # [M, R, N, H] -> [M, R, H, N] once on host
    rel_pos_t = np.ascontiguousarray(
        rel_pos.reshape(M, R, N, H).transpose(0, 1, 3, 2))

    x2_sh, e2_sh = _pmapped(
        w, jnp.asarray(x), jnp.asarray(row_start),
        jnp.asarray(adj2_sh), jnp.asarray(rel_pos_t), jnp.asarray(e_rows),
        jnp.asarray(src_e), jnp.asarray(dstloc_e), jnp.asarray(valid_e),
    )

    x2 = np.asarray(x2_sh).reshape(N, HD).astype(np.float32)
    e2 = np.empty((EG, ED), dtype=np.float32)
    e2_sh = np.asarray(e2_sh)
    for c, ix in enumerate(idx_per_core):
        e2[ix] = e2_sh[c, :len(ix)]

    return x2, e2


# revision 6
# speedup vs baseline: 31.5238x; 1.0491x over previous
"""GTELayer kernel sharded across 8 trn2 NeuronCores.

Strategy (per sharding_hint): shard the dense [N,N,H] score tensor by
destination-node rows (256 rows/core). Softmax over source axis and the
A@V reduction are row-local. Node features / weights are replicated.
Sparse-graph edges are sharded by destination row range on the host
(index prep only); each core gathers sc and scatter-adds pe for its own
rows and produces its slice of e2. GraphNorm-2 global stats are formed
with an 8-way psum of per-core partial sums, so the whole layer is a
single pmap program. Device-side score layout is [i, h, j] (h-major) so
softmax reduces over the contiguous axis and no device transposes of the
[N,N,H] tensor are needed (rel_pos is pre-transposed once on host).
"""

import numpy as np
import jax
import jax.numpy as jnp
from jax import lax

try:
    jax.config.update("jax_compilation_cache_dir", "/tmp/jax_neuron_cache")
    jax.config.update("jax_persistent_cache_min_compile_time_secs", 1.0)
except Exception:
    pass

N = 2048
H = 8
D = 16
HD = 128
ED = 64
FF = 512
EG = 131072
M = 8            # cores
R = N // M       # dst rows per core

_WNAMES = [
    'gn1_w', 'gn1_b', 'gn1_a', 'ln1e_w', 'ln1e_b',
    'Wq', 'bq', 'Wk', 'bk', 'Wv', 'bv', 'Wpe', 'bpe', 'Wap', 'bap',
    'Wo', 'bo', 'Woe', 'boe', 'gn2_w', 'gn2_b', 'gn2_a', 'ln2e_w', 'ln2e_b',
    'Wh1', 'bh1', 'Wh2', 'bh2', 'We1', 'be1', 'We2', 'be2',
]


def _lin(x, W, b):
    return x @ W + b


def _graph_norm(x, w, b, a):
    mu = jnp.mean(x, axis=0, keepdims=True)
    sub = x - a * mu
    var = jnp.mean(sub * sub, axis=0, keepdims=True)
    return w * sub / jnp.sqrt(var + 1e-5) + b


def _layer_norm(x, w, b):
    mu = jnp.mean(x, axis=-1, keepdims=True)
    var = jnp.var(x, axis=-1, keepdims=True)
    return w * (x - mu) / jnp.sqrt(var + 1e-5) + b


def _ffn(x, W1, b1, W2, b2):
    return _lin(jax.nn.gelu(_lin(x, W1, b1), approximate=False), W2, b2)


def _core_fn(w, x, row_start, adj2_rows, rel_pos_t, e_rows,
             src_e, dstloc_e, valid_e):
    """One core: 256 dst rows of attention + its edge slice + node tail."""
    y = _graph_norm(x, w['gn1_w'], w['gn1_b'], w['gn1_a'])
    Q = _lin(y, w['Wq'], w['bq']).reshape(N, H, D)
    K = _lin(y, w['Wk'], w['bk']).reshape(N, H, D)
    V = _lin(y, w['Wv'], w['bv']).reshape(N, H, D)

    xr = lax.dynamic_slice_in_dim(x, row_start, R, axis=0)       # [R,HD]
    Qr = lax.dynamic_slice_in_dim(Q, row_start, R, axis=0)       # [R,H,D]

    en = _layer_norm(e_rows, w['ln1e_w'], w['ln1e_b'])           # [C,ED]
    pe = _lin(en, w['Wpe'], w['bpe'])                            # [C,H]

    # S in [i, h, j] layout: softmax axis is contiguous, no transposes
    S = jnp.einsum('jhd,ihd->ihj', K, Qr) / np.float32(np.sqrt(D))
    S = S * adj2_rows[:, None, :] + rel_pos_t                    # [R,H,N]

    sc = S[dstloc_e, :, src_e]                                   # [C,H]
    e_out = _lin(sc, w['Wap'], w['bap'])                         # [C,ED]
    # set(sc + pe) at unique edges == add(pe); padding adds 0 -> safe dups
    S = S.at[dstloc_e, :, src_e].add(pe * valid_e[:, None])

    A = jax.nn.softmax(jnp.clip(S, -5.0, 5.0), axis=-1)          # [R,H,N]
    wV = jnp.einsum('ihj,jhd->ihd', A, V).reshape(R, HD)
    h_out = _lin(wV, w['Wo'], w['bo'])                           # [R,HD]

    # edge branch tail (fully core-local)
    e_out = _lin(e_out + en, w['Woe'], w['boe'])
    e2 = e_rows + e_out
    e2 = e2 + _ffn(_layer_norm(e2, w['ln2e_w'], w['ln2e_b']),
                   w['We1'], w['be1'], w['We2'], w['be2'])

    # node tail: GraphNorm over ALL nodes via psum of partial sums
    x2 = xr + h_out                                              # [R,HD]
    s1 = lax.psum(jnp.sum(x2, axis=0), axis_name='c')            # [HD]
    s2 = lax.psum(jnp.sum(x2 * x2, axis=0), axis_name='c')       # [HD]
    a = w['gn2_a']
    mu = s1 / np.float32(N)
    var = s2 / np.float32(N) - (2.0 * a - a * a) * mu * mu
    gn = w['gn2_w'] * (x2 - a * mu) / jnp.sqrt(var + 1e-5) + w['gn2_b']
    x2 = x2 + _ffn(gn, w['Wh1'], w['bh1'], w['Wh2'], w['bh2'])
    return x2, e2


_pmapped = jax.pmap(
    _core_fn,
    axis_name='c',
    in_axes=(None, None, 0, 0, 0, 0, 0, 0, 0),
)


def kernel(**inputs):
    x = inputs['x'].astype(np.float32)
    e = inputs['e'].astype(np.float32)
    adj2 = inputs['adj2'].astype(np.float32)
    rel_pos = inputs['rel_pos'].astype(np.float32)
    src = np.asarray(inputs['src'], dtype=np.int32)
    dst = np.asarray(inputs['dst'], dtype=np.int32)
    # Issue the big/async H2D transfers first so they stream while the
    # host does edge-index prep below (device program is unchanged).
    w = {k: jnp.asarray(inputs[k], dtype=jnp.float32) for k in _WNAMES}
    x_d = jnp.asarray(x)
    # [M, R, N, H] -> [M, R, H, N] once on host, then start its upload
    rel_pos_t = np.ascontiguousarray(
        rel_pos.reshape(M, R, N, H).transpose(0, 1, 3, 2))
    rel_pos_d = jnp.asarray(rel_pos_t)
    adj2_d = jnp.asarray(adj2.reshape(M, R, N))

    # --- host-side shard prep (index bookkeeping only) ---
    core_of_edge = dst // R
    idx_per_core = [np.nonzero(core_of_edge == c)[0] for c in range(M)]
    C = max(len(ix) for ix in idx_per_core)
    C = int(-(-C // 128) * 128)  # pad to a multiple of 128

    e_rows = np.zeros((M, C, ED), dtype=np.float32)
    src_e = np.zeros((M, C), dtype=np.int32)
    dstloc_e = np.zeros((M, C), dtype=np.int32)
    valid_e = np.zeros((M, C), dtype=np.float32)
    for c, ix in enumerate(idx_per_core):
        n = len(ix)
        e_rows[c, :n] = e[ix]
        src_e[c, :n] = src[ix]
        dstloc_e[c, :n] = dst[ix] - c * R
        valid_e[c, :n] = 1.0

    row_start = np.arange(M, dtype=np.int32) * R

    x2_sh, e2_sh = _pmapped(
        w, x_d, jnp.asarray(row_start),
        adj2_d, rel_pos_d, jnp.asarray(e_rows),
        jnp.asarray(src_e), jnp.asarray(dstloc_e), jnp.asarray(valid_e),
    )

    x2 = np.asarray(x2_sh).reshape(N, HD).astype(np.float32)
    e2 = np.empty((EG, ED), dtype=np.float32)
    e2_sh = np.asarray(e2_sh)
    for c, ix in enumerate(idx_per_core):
        e2[ix] = e2_sh[c, :len(ix)]

    return x2, e2
